# revision 1
# baseline (speedup 1.0000x reference)
"""Trainium2 Bass kernel for CrossModalFusion.

Computation (per batch b):
  xq  = LayerNorm(dino_query[b] as [HW, C]) ; q = xq @ Wq + bq          [1024, 768]
  xkv = LayerNorm(cogvideo_keys[b] as [TKV, CK]) ; k/v = xkv @ Wk/Wv    [4096, 768]
  12-head cross attention with per-key temporal weights, out proj Wo.

Sharding: 8 cores = 2 batches x 4 head-groups (3 heads / 192 dims each).
Each core computes its batch's attention for its heads plus the partial
output projection; the host sums the 4 partials per batch.

Everything on-device stays feature-major ("transposed"): dino arrives
[C, HW] naturally, scores are computed as S^T [keys, q], AV consumes
P^T directly, and the final output layout [C, H, W] is the natural
result.  The only transposes are xkv (PE-transpose after LayerNorm in
natural layout) and v^T -> v (small per-frame PE transposes).

LayerNorm affine (g, B) is folded into the projection weights on host;
per-row mean/rstd are applied on-device before transposing (kv side) or
via explicit broadcast (q side, stats via ones-matmuls on PE).  The
1/sqrt(64) score scale is folded into Wq; the per-frame temporal weight
becomes the exp() activation's per-partition scale.  Softmax
denominators come for free from a ones-column appended to v.
"""

import numpy as np

B, T, H, W = 2, 4, 32, 32
C = 768              # dino dim
CK = 1920            # cogvideo dim
NH_CORE = 3          # heads per core
HD = 64
DSL = 192            # output dims per core (3 heads x 64)
HWQ = H * W          # 1024 queries
TKV = T * H * W      # 4096 keys
FKV = H * W          # keys per frame = 1024
EPS = 1e-5
P = 128

_CACHE = {}


def _build_program():
    import concourse.bass as bass
    import concourse.bacc as bacc
    import concourse.tile as tile
    from concourse import mybir
    from concourse.masks import make_identity

    f32 = mybir.dt.float32
    f32r = mybir.dt.float32r
    AF = mybir.ActivationFunctionType
    ALU = mybir.AluOpType

    nc = bacc.Bacc("TRN2", target_bir_lowering=False, debug=False)

    # ---- DRAM I/O (per-core tensors; same program on all 8 cores) ----
    d_dino = nc.dram_tensor("dinoT", [C, HWQ], f32r, kind="ExternalInput")
    d_kv = nc.dram_tensor("kv", [TKV, CK], f32r, kind="ExternalInput")
    d_wq = nc.dram_tensor("wq", [C, DSL], f32r, kind="ExternalInput")
    d_wkv = nc.dram_tensor("wkv", [CK, 2 * DSL], f32r, kind="ExternalInput")
    d_wo = nc.dram_tensor("wo", [DSL, C], f32r, kind="ExternalInput")
    d_bq = nc.dram_tensor("bq2", [DSL, 1], f32, kind="ExternalInput")
    d_bkv = nc.dram_tensor("bkv2", [2 * DSL, 1], f32, kind="ExternalInput")
    d_bo = nc.dram_tensor("bo2", [C, 1], f32, kind="ExternalInput")
    d_wf = nc.dram_tensor("wfb", [P, T], f32, kind="ExternalInput")
    d_ones = nc.dram_tensor("onesv", [P, 25], f32r, kind="ExternalInput")
    d_idr = nc.dram_tensor("identr", [P, P], f32r, kind="ExternalInput")
    d_out = nc.dram_tensor("outT", [C, HWQ], f32, kind="ExternalOutput")

    dino_t = d_dino.ap().rearrange("(n p) q -> n p q", p=P)      # [6, 128, 1024]
    kv_t = d_kv.ap().rearrange("(n p) c -> n p c", p=P)          # [32, 128, 1920]
    wq_t = d_wq.ap().rearrange("(n p) d -> n p d", p=P)          # [6, 128, 192]
    wkv_t = d_wkv.ap().rearrange("(n p) d -> n p d", p=P)        # [15, 128, 384]
    out_t = d_out.ap().rearrange("(n p) q -> n p q", p=P)        # [6, 128, 1024]

    NQT = C // P            # 6 q-side c-tiles
    NKC = CK // P           # 15 kv-side c-tiles
    NSP = HWQ // 512        # 2 q spans of 512
    KT_F = FKV // P         # 8 key tiles per frame

    with tile.TileContext(nc) as tc:
        with (
            nc.allow_low_precision(reason="float32r rounding is intentional"),
            tc.tile_pool(name="consts", bufs=1) as consts,
            tc.tile_pool(name="wpool", bufs=1) as wpool,
            tc.tile_pool(name="persist", bufs=1) as persist,
            tc.tile_pool(name="bigio", bufs=4) as bigio,
            tc.tile_pool(name="xtp", bufs=1) as xtp,
            tc.tile_pool(name="kvtp", bufs=2) as kvtp,
            tc.tile_pool(name="vnp", bufs=2) as vnp,
            tc.tile_pool(name="ptp", bufs=2) as ptp,
            tc.tile_pool(name="smalls", bufs=4) as smalls,
            tc.tile_pool(name="outp", bufs=2) as outp,
            tc.tile_pool(name="ps", bufs=2, space="PSUM") as ps,
        ):
            # ---------- constants ----------
            identity = consts.tile([P, P], f32, tag="ident", name="ident")
            make_identity(nc, identity)
            identity_r = consts.tile([P, P], f32r, tag="identr", name="identr")
            nc.sync.dma_start(out=identity_r, in_=d_idr.ap())
            ones_col = consts.tile([P, 1], f32r, tag="ones", name="ones")
            nc.sync.dma_start(out=ones_col, in_=d_ones.ap()[:, 0:1])
            ones_row = consts.tile([1, P], f32r, tag="onesr", name="onesr")
            nc.sync.dma_start(out=ones_row, in_=d_ones.ap().rearrange("p t -> (p t)")[0:P].rearrange("(o p) -> o p", o=1))
            wf_sb = consts.tile([P, T], f32, tag="wf", name="wf")
            nc.sync.dma_start(out=wf_sb, in_=d_wf.ap())
            eps128 = consts.tile([P, 1], f32, tag="eps", name="eps")
            nc.vector.memset(eps128, EPS)

            # ---------- weights ----------
            # dq lives in the (big) xT slot — dead before the kv phase needs it
            dq_all = xtp.tile([P, NQT * HWQ], f32r, tag="xT", name="dqall")
            dq_sb = []
            for i in range(NQT):
                t = dq_all[:, i * HWQ:(i + 1) * HWQ]
                nc.sync.dma_start(out=t, in_=dino_t[i])
                dq_sb.append(t)
            wq_sb = []
            for i in range(NQT):
                t = wpool.tile([P, DSL], f32r, tag=f"wq{i}", name=f"wq{i}")
                nc.sync.dma_start(out=t, in_=wq_t[i])
                wq_sb.append(t)
            wkv_sb = []
            for i in range(NKC):
                t = wpool.tile([P, 2 * DSL], f32r, tag=f"wkv{i}", name=f"wkv{i}")
                nc.sync.dma_start(out=t, in_=wkv_t[i])
                wkv_sb.append(t)
            wo_sb = []
            for i in range(2):  # [128,768] + [64,768]
                pp = P if i == 0 else DSL - P
                t = wpool.tile([pp, C], f32r, tag=f"wo{i}", name=f"wo{i}")
                nc.sync.dma_start(out=t, in_=d_wo.ap()[i * P:i * P + pp, :])
                wo_sb.append(t)
            bq_sb = []
            for i in range(2):
                pp = P if i == 0 else DSL - P
                t = wpool.tile([pp, 1], f32, tag=f"bq{i}", name=f"bq{i}")
                nc.sync.dma_start(out=t, in_=d_bq.ap()[i * P:i * P + pp, :])
                bq_sb.append(t)
            bkv_sb = []
            for i in range(3):
                t = wpool.tile([P, 1], f32, tag=f"bkv{i}", name=f"bkv{i}")
                nc.sync.dma_start(out=t, in_=d_bkv.ap()[i * P:(i + 1) * P, :])
                bkv_sb.append(t)
            bo_sb = []
            for i in range(NQT):
                t = wpool.tile([P, 1], f32, tag=f"bo{i}", name=f"bo{i}")
                nc.sync.dma_start(out=t, in_=d_bo.ap()[i * P:(i + 1) * P, :])
                bo_sb.append(t)

            # ================= Q phase =================
            # dino arrives [C, HW] (feature-major). LN stats over C via
            # ones-matmuls on x and x^2; normalize with broadcast mu/rstd.

            mu_q = smalls.tile([1, HWQ], f32r, tag="muq", name="muq", bufs=1)
            sq_q = smalls.tile([1, HWQ], f32, tag="sqq", name="sqq", bufs=1)
            for kind in range(2):        # 0: sum(x), 1: sum(x^2)
                for s in range(NSP):
                    sl = slice(s * 512, s * 512 + 512)
                    pst = ps.tile([1, 512], f32, tag="proj", name="proj")
                    for i in range(NQT):
                        if kind == 0:
                            src = dq_sb[i][:, sl]
                        else:
                            sqt = smalls.tile([P, 512], f32r, tag="rb", name="sqtile", bufs=1)
                            nc.scalar.activation(sqt, dq_sb[i][:, sl], AF.Square)
                            src = sqt
                        nc.tensor.matmul(
                            pst, ones_col, src,
                            start=(i == 0), stop=(i == NQT - 1))
                    dst = mu_q if kind == 0 else sq_q
                    nc.scalar.activation(dst[:, sl], pst, AF.Copy, scale=1.0 / C)

            # var = E[x^2] - mu^2 ; rstd = 1/sqrt(var + eps)  (in place)
            musq = smalls.tile([1, HWQ], f32r, tag="musq", name="musq", bufs=1)
            nc.scalar.activation(musq, mu_q, AF.Square)
            nc.vector.tensor_sub(musq, sq_q, musq)
            nc.scalar.activation(musq, musq, AF.Sqrt, bias=eps128[0:1, :])
            nc.vector.reciprocal(musq, musq)
            rstd_q = musq

            # broadcast [1, HWQ] -> [P, HWQ] via PE rank-1 outer product
            mu_b = persist.tile([P, HWQ], f32, tag="mub", name="mub")
            rstd_b = persist.tile([P, HWQ], f32, tag="rstdb", name="rstdb")
            for src, dstb in ((mu_q, mu_b), (rstd_q, rstd_b)):
                for s in range(NSP):
                    sl = slice(s * 512, s * 512 + 512)
                    pb = ps.tile([P, 512], f32, tag="proj", name="bc")
                    nc.tensor.matmul(pb, ones_row,
                                     src[:, sl],
                                     start=True, stop=True)
                    nc.vector.tensor_copy(dstb[:, sl], pb)

            for i in range(NQT):
                nc.vector.tensor_sub(dq_sb[i], dq_sb[i], mu_b)
                nc.vector.tensor_mul(dq_sb[i], dq_sb[i], rstd_b)

            # q projection: qT[d, q] = wq.T @ xnq  (+bias)
            qT = []
            for mi in range(2):
                pp = P if mi == 0 else DSL - P
                qt = persist.tile([pp, HWQ], f32r, tag=f"qT{mi}", name=f"qT{mi}")
                for s in range(NSP):
                    sl = slice(s * 512, s * 512 + 512)
                    pq = ps.tile([pp, 512], f32, tag="proj", name="proj")
                    for i in range(NQT):
                        nc.tensor.matmul(
                            pq, wq_sb[i][:, mi * P:mi * P + pp],
                            dq_sb[i][:, sl],
                            start=(i == 0), stop=(i == NQT - 1))
                    nc.scalar.activation(qt[:, sl], pq, AF.Identity,
                                         bias=bq_sb[mi])
                qT.append(qt)

            # ================= KV + attention, per frame =================
            # av accumulators [65, HWQ] per head (row 64 = softmax denom)
            av_sb = []
            for h in range(NH_CORE):
                t = persist.tile([HD + 1, HWQ], f32, tag=f"av{h}", name=f"av{h}")
                nc.vector.memset(t, 0.0)
                av_sb.append(t)

            def emit_ln(f, kvT, vn):
                """Chunk thunks for LayerNorm+transpose+proj+v of frame f."""
                chunks = []
                for sp in range(2):
                    def open_span(sp=sp):
                        xT = xtp.tile([P, NKC * 512], f32r, tag="xT",
                                      name="xT")
                        return xT, xT.rearrange("p (c q) -> p c q", q=512)
                    span_state = {}

                    def rowtile(sp=sp, rt=None, span_state=span_state):
                        if rt == 0:
                            span_state['xT'], span_state['xT3'] = open_span()
                        xT3 = span_state['xT3']
                        gi = f * 8 + sp * 4 + rt
                        xin = bigio.tile([P, CK], f32r, tag="kvin",
                                         name="kvin")
                        nc.sync.dma_start(out=xin, in_=kv_t[gi])
                        stats = smalls.tile([P, 4, 6], f32, tag="bnst",
                                            name="bnst")
                        for ch in range(4):
                            nc.vector.bn_stats(stats[:, ch, :],
                                               xin[:, ch * 480:(ch + 1) * 480])
                        mv = smalls.tile([P, 2], f32, tag="bnmv", name="bnmv")
                        nc.vector.bn_aggr(mv, stats)
                        rstd = smalls.tile([P, 1], f32, tag="rstd",
                                           name="rstd")
                        nc.scalar.activation(rstd, mv[:, 1:2], AF.Ln,
                                             bias=eps128)
                        nc.scalar.activation(rstd, rstd, AF.Exp, scale=-0.5)
                        nc.gpsimd.tensor_scalar(
                            xin, xin, mv[:, 0:1], rstd,
                            op0=ALU.subtract, op1=ALU.mult)
                        for g in range(4):
                            c0 = g * 4
                            ng = min(4, NKC - c0)
                            ptb = ps.tile([P, 512], f32r, tag="tr", name="tr")
                            for j in range(ng):
                                nc.tensor.transpose(
                                    ptb[:, j * P:(j + 1) * P],
                                    xin[:, (c0 + j) * P:(c0 + j + 1) * P],
                                    identity_r)
                            src3 = ptb[:, 0:ng * P].rearrange(
                                "p (c q) -> p c q", q=P)
                            dst3 = xT3[:, c0:c0 + ng, rt * P:(rt + 1) * P]
                            if (rt + g) % 4 == 0:
                                nc.scalar.copy(dst3, src3)
                            else:
                                nc.vector.tensor_copy(dst3, src3)
                    for rt in range(4):
                        chunks.append(lambda rt=rt, fn=rowtile: fn(rt=rt))

                    def proj(mi, sp=sp, span_state=span_state):
                        xT = span_state['xT']
                        pq = ps.tile([P, 512], f32, tag="proj", name="proj")
                        for ci in range(NKC):
                            nc.tensor.matmul(
                                pq, wkv_sb[ci][:, mi * P:(mi + 1) * P],
                                xT[:, ci * 512:(ci + 1) * 512],
                                start=(ci == 0), stop=(ci == NKC - 1))
                        nc.vector.tensor_scalar_add(
                            kvT[mi][:, sp * 512:(sp + 1) * 512], pq,
                            bkv_sb[mi])
                    for mi in range(3):
                        chunks.append(lambda mi=mi, fn=proj: fn(mi))

                def vtr(kt):
                    vt = ps.tile([P, 2, P], f32r, tag="tr", name="vt")
                    for j in (1, 2):
                        nc.tensor.transpose(
                            vt[:, j - 1, :],
                            kvT[j][:, kt * P:(kt + 1) * P], identity_r)
                    s3 = vt.rearrange("p a b -> p (a b)")[:, HD:HD + DSL]
                    s3 = s3.rearrange("p (a b) -> p a b", a=NH_CORE)
                    if kt % 2 == 0:
                        nc.scalar.copy(vn[:, kt, :, 0:HD], s3)
                    else:
                        nc.vector.tensor_copy(vn[:, kt, :, 0:HD], s3)
                for kt in range(KT_F):
                    chunks.append(lambda kt=kt: vtr(kt))

                def ones():
                    nc.sync.dma_start(
                        out=vn[:, :, :, HD],
                        in_=d_ones.ap()[:, 0:KT_F * NH_CORE].rearrange(
                            "p (a b) -> p a b", a=KT_F))
                chunks.append(ones)
                return chunks

            def emit_att(f, kvT, vn):
                """Chunk thunks for scores+exp+AV of frame f."""
                hsl = [(kvT[0][0:HD, :], qT[0][0:HD, :]),
                       (kvT[0][HD:P, :], qT[0][HD:P, :]),
                       (kvT[1][0:HD, :], qT[1][0:HD, :])]
                chunks = []
                for s in range(NSP):
                    sl = slice(s * 512, s * 512 + 512)
                    for hs in ((0, 1), (2,)):
                        state = {}

                        def ktchunk(kt, hs=hs, sl=sl, state=state):
                            if kt == 0:
                                state['avps'] = {
                                    h: ps.tile([HD + 1, 512], f32, tag="av",
                                               name="av") for h in hs}
                            avps = state['avps']
                            pes = {}
                            for h in hs:
                                kT_h, q_h = hsl[h]
                                sc = ps.tile([P, 512], f32, tag="score",
                                             name="score")
                                nc.tensor.matmul(
                                    sc, kT_h[:, kt * P:(kt + 1) * P],
                                    q_h[:, sl], start=True, stop=True)
                                pe = ptp.tile([P, 512], f32r, tag="pt",
                                              name="pt")
                                nc.scalar.activation(pe, sc, AF.Exp,
                                                     scale=wf_sb[:, f:f + 1])
                                pes[h] = pe
                            for h in hs:
                                nc.tensor.matmul(
                                    avps[h], vn[:, kt, h, :], pes[h],
                                    start=(kt == 0), stop=(kt == KT_F - 1))

                        def adds(hs=hs, sl=sl, state=state):
                            for h in hs:
                                nc.vector.tensor_add(
                                    av_sb[h][:, sl], av_sb[h][:, sl],
                                    state['avps'][h])
                        for kt in range(KT_F):
                            chunks.append(lambda kt=kt, fn=ktchunk: fn(kt))
                        chunks.append(adds)
                return chunks

            # software pipeline: LN of frame f interleaved with attention
            # of frame f-1 so the static PE stream can fill exp-wait gaps
            prev_att = []
            for f in range(T):
                kvT = [kvtp.tile([P, FKV], f32r, tag=f"kvT{m}",
                                 name=f"kvT{m}") for m in range(3)]
                vn = vnp.tile([P, KT_F, NH_CORE, HD + 1], f32r, tag="vnat",
                              name="vnat")
                ln_chunks = emit_ln(f, kvT, vn)
                na, nl = len(prev_att), len(ln_chunks)
                ai = 0
                for li in range(nl):
                    ln_chunks[li]()
                    want = (li + 1) * na // nl
                    while ai < want:
                        prev_att[ai]()
                        ai += 1
                prev_att = emit_att(f, kvT, vn)
            for c in prev_att:
                c()

            # ================= finale =================
            # normalize each head by its denominator, assemble attnT
            attnT = [persist.tile([P, HWQ], f32r, tag="mub", name="mub"),
                     persist.tile([DSL - P, HWQ], f32r, tag="rstdb", name="rstdb")]
            for h in range(NH_CORE):
                rcp = smalls.tile([1, HWQ], f32r, tag="rcp", name="rcp", bufs=2)
                nc.vector.reciprocal(rcp, av_sb[h][HD:HD + 1, :])
                rb = smalls.tile([HD, HWQ], f32, tag="rb", name="rb", bufs=1)
                for s in range(NSP):
                    sl = slice(s * 512, s * 512 + 512)
                    pb = ps.tile([HD, 512], f32, tag="proj", name="bcr")
                    nc.tensor.matmul(pb, ones_row[:, 0:HD],
                                     rcp[:, sl],
                                     start=True, stop=True)
                    nc.scalar.copy(rb[:, sl], pb)
                dst = attnT[0][:HD, :] if h == 0 else (
                    attnT[0][HD:P, :] if h == 1 else attnT[1][0:HD, :])
                nc.vector.tensor_mul(dst, av_sb[h][0:HD, :], rb)

            # out projection: outT = wo.T @ attnT (+bo)
            for mi in range(NQT):
                ot = outp.tile([P, HWQ], f32, tag="ot", name="ot")
                for s in range(NSP):
                    sl = slice(s * 512, s * 512 + 512)
                    pq = ps.tile([P, 512], f32, tag="proj", name="proj")
                    for ki in range(2):
                        kp = P if ki == 0 else DSL - P
                        nc.tensor.matmul(
                            pq, wo_sb[ki][:, mi * P:(mi + 1) * P],
                            attnT[ki][:, sl],
                            start=(ki == 0), stop=(ki == 1))
                    nc.scalar.activation(ot[:, sl], pq, AF.Identity,
                                         bias=bo_sb[mi])
                nc.sync.dma_start(out=out_t[mi], in_=ot)

    nc.compile()
    return nc


def _prep_core_inputs(inputs, b, hg):
    sl = slice(DSL * hg, DSL * hg + DSL)
    gq = inputs["gq"].astype(np.float64)
    Bq = inputs["Bq"].astype(np.float64)
    gkv = inputs["gkv"].astype(np.float64)
    Bkv = inputs["Bkv"].astype(np.float64)
    Wq = inputs["Wq"].astype(np.float64)
    Wk = inputs["Wk"].astype(np.float64)
    Wv = inputs["Wv"].astype(np.float64)

    wq2 = ((gq[:, None] * Wq[:, sl]) / 8.0).astype(np.float32)
    bq2 = ((Bq @ Wq[:, sl] + inputs["bq"][sl]) / 8.0).astype(np.float32)
    wk2 = (gkv[:, None] * Wk[:, sl]).astype(np.float32)
    bk2 = (Bkv @ Wk[:, sl] + inputs["bk"][sl]).astype(np.float32)
    wv2 = (gkv[:, None] * Wv[:, sl]).astype(np.float32)
    bv2 = (Bkv @ Wv[:, sl] + inputs["bv"][sl]).astype(np.float32)

    bo2 = inputs["bo"] if hg == 0 else np.zeros(C, np.float32)
    wfb = np.tile(np.asarray(inputs["temporal_weights"])[None, :],
                  (P, 1)).astype(np.float32)

    return {
        "dinoT": np.ascontiguousarray(
            np.asarray(inputs["dino_query"])[b].reshape(C, HWQ)),
        "kv": np.ascontiguousarray(
            np.asarray(inputs["cogvideo_keys"])[b].reshape(TKV, CK)),
        "wq": wq2,
        "wkv": np.ascontiguousarray(np.concatenate([wk2, wv2], axis=1)),
        "wo": np.ascontiguousarray(np.asarray(inputs["Wo"])[sl, :]),
        "bq2": bq2.reshape(DSL, 1),
        "bkv2": np.concatenate([bk2, bv2]).reshape(2 * DSL, 1),
        "bo2": np.ascontiguousarray(np.asarray(bo2).reshape(C, 1)),
        "wfb": wfb,
        "onesv": np.ones((P, 25), np.float32),
        "identr": np.eye(P, dtype=np.float32),
    }


def kernel(**inputs):
    from concourse.bass_utils import run_bass_kernel_spmd

    if "nc" not in _CACHE:
        _CACHE["nc"] = _build_program()
    nc = _CACHE["nc"]

    in_maps = []
    for core in range(8):
        b, hg = core // 4, core % 4
        in_maps.append(_prep_core_inputs(inputs, b, hg))

    res = run_bass_kernel_spmd(nc, in_maps, list(range(8)),
                               trace=bool(_CACHE.get("profile")))
    _CACHE["last_result"] = res

    out = np.zeros((B, C, H, W), np.float32)
    for b in range(B):
        acc = np.zeros((C, HWQ), np.float32)
        for hg in range(4):
            acc += res.results[b * 4 + hg]["outT"]
        out[b] = acc.reshape(C, H, W)
    return out

